# revision 9
# baseline (speedup 1.0000x reference)
"""Trainium2 Bass kernel for nn_CrossAttention (B=8, C=256, W=H=64) — v2.

Sharding: data-parallel over batch across the 8 NeuronCores; the small 1x1
conv weights and gamma are replicated (host-side prep replicates Wf/Wg into
the 4 PE row-bands so the f/g matmuls land pre-replicated).

Per-core computation (one batch, xf = x reshaped [C, N], N = W*H = 4096):
    f   = Wf @ xf + bf       [CQ, N]  stored 4x band-replicated [128, N] bf16
    g   = Wg @ yf + bg       [CQ, N]  same
    hhT = (Wh @ xf).T        [N, C]   bf16, + ones column at c=C (denominator)
    LT[j, i]  = sum_d g[d, j] f[d, i]    (pair tiles [128, 2, IC] in psum)
    E = exp(LT)              one ACT call per pair (N=1024), bf16
    UT[i, c] = sum_j E[j, i] * hhT[j, c]  -> col C holds D[i] = sum_j E[j, i]
    satT[i, c] = UT[i, c] * (1/D[i]) * gamma
    sa[c, i] = DMA-xbar-transpose(satT)
    out[c, i] = sa + gamma*bh[c] + x[c, i]   (bias folds out: rows of attn sum to 1)
"""

import numpy as np

import concourse.bass as bass
import concourse.mybir as mybir
import concourse.tile as tile
from concourse import bacc
from concourse.bass import ds, ts
from concourse.bass_utils import run_bass_kernel_spmd

FP32 = mybir.dt.float32
BF16 = mybir.dt.bfloat16
AF = mybir.ActivationFunctionType
ALU = mybir.AluOpType

C = 256
CQ = 32
N_CORES = 8


def build_nc(n=4096):
    assert n % 256 == 0
    NB = n // 128            # j blocks
    NPAIR = NB // 2
    IC = min(512, n)         # i-chunk
    NIC = n // IC
    NQ = IC // 128           # 128-row i tiles per chunk
    DCH = min(512, n)        # input DMA chunk
    NCH = n // DCH

    nc = bacc.Bacc("TRN2", target_bir_lowering=False, debug=False)

    x_d = nc.dram_tensor("x", [C, n], FP32, kind="ExternalInput").ap()
    y_d = nc.dram_tensor("y", [C, n], FP32, kind="ExternalInput").ap()
    wf_d = nc.dram_tensor("wfT_rep", [C, 128], FP32, kind="ExternalInput").ap()
    wg_d = nc.dram_tensor("wgT_rep", [C, 128], FP32, kind="ExternalInput").ap()
    wh_d = nc.dram_tensor("whT", [C, C], FP32, kind="ExternalInput").ap()
    bf_d = nc.dram_tensor("bf_rep", [128, 1], FP32, kind="ExternalInput").ap()
    bg_d = nc.dram_tensor("bg_rep", [128, 1], FP32, kind="ExternalInput").ap()
    bh_d = nc.dram_tensor("bh", [C, 1], FP32, kind="ExternalInput").ap()
    gamma_d = nc.dram_tensor("gamma", [1, 1], FP32, kind="ExternalInput").ap()
    out_d = nc.dram_tensor("out", [C, n], FP32, kind="ExternalOutput").ap()

    with tile.TileContext(nc) as tc:
        with tc.tile_pool(name="persist", bufs=1) as persist, \
             tc.tile_pool(name="consts", bufs=1) as consts, \
             tc.tile_pool(name="ystage", bufs=2) as ystage, \
             tc.tile_pool(name="prod", bufs=4, space="PSUM") as prod, \
             tc.tile_pool(name="ut", bufs=4, space="PSUM") as utp, \
             tc.tile_pool(name="ex", bufs=6) as epool, \
             tc.tile_pool(name="tail", bufs=8) as tailp, \
             tc.tile_pool(name="sa", bufs=2) as sap, \
             tc.tile_pool(name="stage", bufs=3) as stgp:
            # ---- persistent SBUF tensors -------------------------------
            x_sb = persist.tile([128, 2, n], FP32, tag="x_sb")
            xb = persist.tile([128, 2, n], BF16, tag="xb")
            yb = persist.tile([128, 2, n], BF16, tag="yb")
            f_rep = persist.tile([128, n], BF16, tag="f_rep")
            g_rep = persist.tile([128, n], BF16, tag="g_rep")
            hhT = persist.tile([128, NB, C + 1], BF16, tag="hhT")

            wf_f = consts.tile([128, 2, 128], FP32, tag="wf_f")
            wg_f = consts.tile([128, 2, 128], FP32, tag="wg_f")
            wh_f = consts.tile([128, 2, C], FP32, tag="wh_f")
            wf_b = consts.tile([128, 2, 128], BF16, tag="wf_b")
            wg_b = consts.tile([128, 2, 128], BF16, tag="wg_b")
            wh_b = consts.tile([128, 2, C], BF16, tag="wh_b")
            bf_sb = consts.tile([128, 1], FP32, tag="bf_sb")
            bg_sb = consts.tile([128, 1], FP32, tag="bg_sb")
            bh_sb = consts.tile([128, 2, 1], FP32, tag="bh_sb")
            gbh = consts.tile([128, 2, 1], FP32, tag="gbh")
            gamma_sb = consts.tile([128, 1], FP32, tag="gamma_sb")

            # ---- weight DMAs + casts (small, first) --------------------
            for cb in range(2):
                nc.sync.dma_start(out=wf_f[:, cb, :], in_=wf_d[cb * 128:(cb + 1) * 128, :])
                nc.sync.dma_start(out=wg_f[:, cb, :], in_=wg_d[cb * 128:(cb + 1) * 128, :])
                nc.sync.dma_start(out=wh_f[:, cb, :], in_=wh_d[cb * 128:(cb + 1) * 128, :])
                nc.sync.dma_start(out=bh_sb[:, cb, :], in_=bh_d[cb * 128:(cb + 1) * 128, :])
            nc.sync.dma_start(out=bf_sb, in_=bf_d[:, :])
            nc.sync.dma_start(out=bg_sb, in_=bg_d[:, :])
            nc.sync.dma_start(out=gamma_sb, in_=gamma_d[:, :].to_broadcast([128, 1]))
            for cb in range(2):
                nc.vector.tensor_copy(wf_b[:, cb, :], wf_f[:, cb, :])
                nc.vector.tensor_copy(wg_b[:, cb, :], wg_f[:, cb, :])
                nc.vector.tensor_copy(wh_b[:, cb, :], wh_f[:, cb, :])
            nc.vector.tensor_scalar_mul(gbh, bh_sb, gamma_sb)
            nc.vector.memset(hhT[:, :, C:C + 1], 1.0)

            # ---- phase A: chunked input DMA + casts + f/g/hh -----------
            for ch in range(NCH):
                ccols = ts(ch, DCH)
                ystg = ystage.tile([128, 2, DCH], FP32, tag="ystg")
                for cb in range(2):
                    nc.sync.dma_start(out=x_sb[:, cb, ccols],
                                      in_=x_d[cb * 128:(cb + 1) * 128, ccols])
                    nc.sync.dma_start(out=ystg[:, cb, :],
                                      in_=y_d[cb * 128:(cb + 1) * 128, ccols])
                for cb in range(2):
                    nc.vector.tensor_copy(xb[:, cb, ccols], x_sb[:, cb, ccols])
                    nc.vector.tensor_copy(yb[:, cb, ccols], ystg[:, cb, :])
                # f/g in 512-wide subchunks, band-replicated via wf_b/wg_b
                for s in range(DCH // 512):
                    scols = ds(ch * DCH + s * 512, 512)
                    pf = prod.tile([128, 512], FP32, tag="prod", name="pf")
                    for cb in range(2):
                        nc.tensor.matmul(pf, lhsT=wf_b[:, cb, :],
                                         rhs=xb[:, cb, scols],
                                         start=(cb == 0), stop=(cb == 1))
                    nc.vector.tensor_scalar_add(f_rep[:, scols], pf, bf_sb)
                    pg = prod.tile([128, 512], FP32, tag="prod", name="pg")
                    for cb in range(2):
                        nc.tensor.matmul(pg, lhsT=wg_b[:, cb, :],
                                         rhs=yb[:, cb, scols],
                                         start=(cb == 0), stop=(cb == 1))
                    nc.vector.tensor_scalar_add(g_rep[:, scols], pg, bg_sb)
                # hhT for the j-blocks of this chunk
                for pp in range(DCH // 128):
                    jb = ch * (DCH // 128) + pp
                    ph = prod.tile([128, 512], FP32, tag="prod", name="ph")
                    for cb in range(2):
                        nc.tensor.matmul(ph[:, 0:C],
                                         lhsT=xb[:, cb, ts(jb, 128)],
                                         rhs=wh_b[:, cb, :],
                                         start=(cb == 0), stop=(cb == 1))
                    nc.vector.tensor_copy(hhT[:, jb, 0:C], ph[:, 0:C])

            # ---- main attention loop -----------------------------------
            # Software-pipelined: per group jg of 4 j-blocks, emit logits(jg)
            # and exp(jg), then the UT matmuls of group jg-1 — so the PE works
            # on UT(jg-1) while ACT computes exp(jg).
            NG = NB // 4
            for ic in range(NIC):
                icols = ds(ic * IC, IC)
                uts = [utp.tile([128, C + 1], FP32, tag="ut", name=f"ut{q}")
                       for q in range(NQ)]
                saT = sap.tile([128, 2, IC], BF16, tag="saT")
                prevE = None

                def emit_ut(jg, Es):
                    for q in range(NQ):
                        for b in range(4):
                            j = jg * 4 + b
                            nc.tensor.matmul(
                                uts[q],
                                lhsT=Es[b][:, ds(q * 128, 128)],
                                rhs=hhT[:, j, :],
                                start=(j == 0), stop=(j == NB - 1),
                                skip_group_check=True)

                for jg in range(NG):
                    Es = []
                    for b in range(4):
                        j = jg * 4 + b
                        lt = prod.tile([128, IC], FP32, tag="prod",
                                       name=f"lt{b}")
                        # full-K matmul over the 4 replicated bands => 4*L;
                        # the /4 is folded into the exp's free scale.
                        nc.tensor.matmul(
                            lt,
                            lhsT=g_rep[:, ts(j, 128)],
                            rhs=f_rep[:, icols],
                            start=True, stop=True)
                        E = epool.tile([128, IC], BF16, tag="ex", name=f"ex{b}")
                        nc.scalar.activation(E, lt, AF.Exp, scale=0.25)
                        Es.append(E)
                    if prevE is not None:
                        emit_ut(jg - 1, prevE)
                    prevE = Es
                emit_ut(NG - 1, prevE)
                # tail: normalize, transpose via DMA xbar, residual add, store
                for q in range(NQ):
                    rd = tailp.tile([128, 1], FP32, tag="rd")
                    nc.vector.reciprocal(rd, uts[q][:, C:C + 1])
                    satT = tailp.tile([128, C], BF16, tag="satT")
                    nc.vector.tensor_scalar(satT, uts[q][:, 0:C], rd,
                                            gamma_sb,
                                            op0=ALU.mult, op1=ALU.mult)
                    for cb in range(2):
                        nc.sync.dma_start(out=saT[:, cb, ds(q * 128, 128)],
                                          in_=satT[:, ds(cb * 128, 128)],
                                          transpose=True)
                stage = stgp.tile([128, 2, IC], FP32, tag="stage")
                if ic == NIC - 1:
                    for cb in range(2):
                        for q in range(NQ):
                            qc = ds(q * 128, 128)
                            nc.vector.tensor_scalar_add(stage[:, cb, qc],
                                                        saT[:, cb, qc],
                                                        gbh[:, cb, :])
                            nc.vector.tensor_add(
                                stage[:, cb, qc], stage[:, cb, qc],
                                x_sb[:, cb, ds(ic * IC + q * 128, 128)])
                            nc.sync.dma_start(
                                out=out_d[cb * 128:(cb + 1) * 128,
                                          ds(ic * IC + q * 128, 128)],
                                in_=stage[:, cb, qc])
                else:
                    for cb in range(2):
                        nc.vector.tensor_scalar_add(stage[:, cb, :],
                                                    saT[:, cb, :],
                                                    gbh[:, cb, :])
                        nc.vector.tensor_add(stage[:, cb, :], stage[:, cb, :],
                                             x_sb[:, cb, icols])
                        nc.sync.dma_start(out=out_d[cb * 128:(cb + 1) * 128,
                                                    icols],
                                          in_=stage[:, cb, :])

    nc.compile()
    return nc


_NC_CACHE = {}


def _get_nc(n=4096):
    if n not in _NC_CACHE:
        _NC_CACHE[n] = build_nc(n)
    return _NC_CACHE[n]


def make_in_maps(x, y, Wf, bf, Wg, bg, Wh, bh, gamma):
    x = np.asarray(x, dtype=np.float32)
    y = np.asarray(y, dtype=np.float32)
    B, C_, W_, H_ = x.shape
    n = W_ * H_
    wf_rep = np.ascontiguousarray(np.tile(np.asarray(Wf, np.float32).T, (1, 4)))
    wg_rep = np.ascontiguousarray(np.tile(np.asarray(Wg, np.float32).T, (1, 4)))
    whT = np.ascontiguousarray(np.asarray(Wh, np.float32).T)
    bf_rep = np.ascontiguousarray(np.tile(np.asarray(bf, np.float32), 4).reshape(128, 1))
    bg_rep = np.ascontiguousarray(np.tile(np.asarray(bg, np.float32), 4).reshape(128, 1))
    bh_ = np.asarray(bh, np.float32).reshape(C_, 1)
    gm_ = np.asarray(gamma, np.float32).reshape(1, 1)
    in_maps = []
    for b in range(B):
        in_maps.append({
            "x": np.ascontiguousarray(x[b].reshape(C_, n)),
            "y": np.ascontiguousarray(y[b].reshape(C_, n)),
            "wfT_rep": wf_rep, "wgT_rep": wg_rep, "whT": whT,
            "bf_rep": bf_rep, "bg_rep": bg_rep, "bh": bh_, "gamma": gm_,
        })
    return in_maps, (B, C_, W_, H_)


def run_spmd(inputs: dict, trace: bool = False):
    """Run the SPMD kernel; returns (out [B,C,W,H], BassKernelResults)."""
    in_maps, (B, C_, W_, H_) = make_in_maps(**inputs)
    nc = _get_nc(W_ * H_)
    res = run_bass_kernel_spmd(nc, in_maps, core_ids=list(range(B)), trace=trace)
    out = np.stack([res.results[b]["out"].reshape(C_, W_, H_) for b in range(B)])
    return np.ascontiguousarray(out, dtype=np.float32), res


def kernel(x, y, Wf, bf, Wg, bg, Wh, bh, gamma):
    out, _ = run_spmd(dict(x=x, y=y, Wf=Wf, bf=bf, Wg=Wg, bg=bg,
                           Wh=Wh, bh=bh, gamma=gamma))
    return out


# revision 10
# speedup vs baseline: 1.0235x; 1.0235x over previous
"""Trainium2 Bass kernel for nn_CrossAttention (B=8, C=256, W=H=64) — v2.

Sharding: data-parallel over batch across the 8 NeuronCores; the small 1x1
conv weights and gamma are replicated (host-side prep replicates Wf/Wg into
the 4 PE row-bands so the f/g matmuls land pre-replicated).

Per-core computation (one batch, xf = x reshaped [C, N], N = W*H = 4096):
    f   = Wf @ xf + bf       [CQ, N]  stored 4x band-replicated [128, N] bf16
    g   = Wg @ yf + bg       [CQ, N]  same
    hhT = (Wh @ xf).T        [N, C]   bf16, + ones column at c=C (denominator)
    LT[j, i]  = sum_d g[d, j] f[d, i]    (pair tiles [128, 2, IC] in psum)
    E = exp(LT)              one ACT call per pair (N=1024), bf16
    UT[i, c] = sum_j E[j, i] * hhT[j, c]  -> col C holds D[i] = sum_j E[j, i]
    satT[i, c] = UT[i, c] * (1/D[i]) * gamma
    sa[c, i] = DMA-xbar-transpose(satT)
    out[c, i] = sa + gamma*bh[c] + x[c, i]   (bias folds out: rows of attn sum to 1)
"""

import numpy as np

import concourse.bass as bass
import concourse.mybir as mybir
import concourse.tile as tile
from concourse import bacc
from concourse.bass import ds, ts
from concourse.bass_utils import run_bass_kernel_spmd

FP32 = mybir.dt.float32
BF16 = mybir.dt.bfloat16
AF = mybir.ActivationFunctionType
ALU = mybir.AluOpType

C = 256
CQ = 32
N_CORES = 8


def build_nc(n=4096):
    assert n % 256 == 0
    NB = n // 128            # j blocks
    NPAIR = NB // 2
    IC = min(512, n)         # i-chunk
    NIC = n // IC
    NQ = IC // 128           # 128-row i tiles per chunk
    DCH = min(1024, n)       # input DMA chunk
    NCH = n // DCH

    nc = bacc.Bacc("TRN2", target_bir_lowering=False, debug=False)

    x_d = nc.dram_tensor("x", [C, n], FP32, kind="ExternalInput").ap()
    y_d = nc.dram_tensor("y", [C, n], FP32, kind="ExternalInput").ap()
    wf_d = nc.dram_tensor("wfT_rep", [C, 128], FP32, kind="ExternalInput").ap()
    wg_d = nc.dram_tensor("wgT_rep", [C, 128], FP32, kind="ExternalInput").ap()
    wh_d = nc.dram_tensor("whT", [C, C], FP32, kind="ExternalInput").ap()
    bf_d = nc.dram_tensor("bf_rep", [128, 1], FP32, kind="ExternalInput").ap()
    bg_d = nc.dram_tensor("bg_rep", [128, 1], FP32, kind="ExternalInput").ap()
    bh_d = nc.dram_tensor("bh", [C, 1], FP32, kind="ExternalInput").ap()
    gamma_d = nc.dram_tensor("gamma", [1, 1], FP32, kind="ExternalInput").ap()
    out_d = nc.dram_tensor("out", [C, n], FP32, kind="ExternalOutput").ap()

    with tile.TileContext(nc) as tc:
        with tc.tile_pool(name="persist", bufs=1) as persist, \
             tc.tile_pool(name="consts", bufs=1) as consts, \
             tc.tile_pool(name="ystage", bufs=2) as ystage, \
             tc.tile_pool(name="prod", bufs=4, space="PSUM") as prod, \
             tc.tile_pool(name="ut", bufs=4, space="PSUM") as utp, \
             tc.tile_pool(name="ex", bufs=6) as epool, \
             tc.tile_pool(name="tail", bufs=8) as tailp, \
             tc.tile_pool(name="sa", bufs=2) as sap, \
             tc.tile_pool(name="stage", bufs=3) as stgp:
            # ---- persistent SBUF tensors -------------------------------
            x_sb = persist.tile([128, 2, n], FP32, tag="x_sb")
            xb = persist.tile([128, 2, n], BF16, tag="xb")
            yb = persist.tile([128, 2, n], BF16, tag="yb")
            f_rep = persist.tile([128, n], BF16, tag="f_rep")
            g_rep = persist.tile([128, n], BF16, tag="g_rep")
            hhT = persist.tile([128, NB, C + 1], BF16, tag="hhT")

            wf_f = consts.tile([128, 2, 128], FP32, tag="wf_f")
            wg_f = consts.tile([128, 2, 128], FP32, tag="wg_f")
            wh_f = consts.tile([128, 2, C], FP32, tag="wh_f")
            wf_b = consts.tile([128, 2, 128], BF16, tag="wf_b")
            wg_b = consts.tile([128, 2, 128], BF16, tag="wg_b")
            wh_b = consts.tile([128, 2, C], BF16, tag="wh_b")
            bf_sb = consts.tile([128, 1], FP32, tag="bf_sb")
            bg_sb = consts.tile([128, 1], FP32, tag="bg_sb")
            bh_sb = consts.tile([128, 2, 1], FP32, tag="bh_sb")
            gbh = consts.tile([128, 2, 1], FP32, tag="gbh")
            gamma_sb = consts.tile([128, 1], FP32, tag="gamma_sb")

            # ---- weight DMAs + casts (small, first) --------------------
            for cb in range(2):
                nc.sync.dma_start(out=wf_f[:, cb, :], in_=wf_d[cb * 128:(cb + 1) * 128, :])
                nc.sync.dma_start(out=wg_f[:, cb, :], in_=wg_d[cb * 128:(cb + 1) * 128, :])
                nc.sync.dma_start(out=wh_f[:, cb, :], in_=wh_d[cb * 128:(cb + 1) * 128, :])
                nc.sync.dma_start(out=bh_sb[:, cb, :], in_=bh_d[cb * 128:(cb + 1) * 128, :])
            nc.sync.dma_start(out=bf_sb, in_=bf_d[:, :])
            nc.sync.dma_start(out=bg_sb, in_=bg_d[:, :])
            nc.sync.dma_start(out=gamma_sb, in_=gamma_d[:, :].to_broadcast([128, 1]))
            for cb in range(2):
                nc.vector.tensor_copy(wf_b[:, cb, :], wf_f[:, cb, :])
                nc.vector.tensor_copy(wg_b[:, cb, :], wg_f[:, cb, :])
                nc.vector.tensor_copy(wh_b[:, cb, :], wh_f[:, cb, :])
            nc.vector.tensor_scalar_mul(gbh, bh_sb, gamma_sb)
            nc.vector.memset(hhT[:, :, C:C + 1], 1.0)

            # ---- phase A: chunked input DMA + casts + f/g/hh -----------
            for ch in range(NCH):
                ccols = ts(ch, DCH)
                ystg = ystage.tile([128, 2, DCH], FP32, tag="ystg")
                for cb in range(2):
                    nc.sync.dma_start(out=x_sb[:, cb, ccols],
                                      in_=x_d[cb * 128:(cb + 1) * 128, ccols])
                    nc.sync.dma_start(out=ystg[:, cb, :],
                                      in_=y_d[cb * 128:(cb + 1) * 128, ccols])
                for cb in range(2):
                    nc.vector.tensor_copy(xb[:, cb, ccols], x_sb[:, cb, ccols])
                    nc.vector.tensor_copy(yb[:, cb, ccols], ystg[:, cb, :])
                # f/g in 512-wide subchunks, band-replicated via wf_b/wg_b
                for s in range(DCH // 512):
                    scols = ds(ch * DCH + s * 512, 512)
                    pf = prod.tile([128, 512], FP32, tag="prod", name="pf")
                    for cb in range(2):
                        nc.tensor.matmul(pf, lhsT=wf_b[:, cb, :],
                                         rhs=xb[:, cb, scols],
                                         start=(cb == 0), stop=(cb == 1))
                    nc.vector.tensor_scalar_add(f_rep[:, scols], pf, bf_sb)
                    pg = prod.tile([128, 512], FP32, tag="prod", name="pg")
                    for cb in range(2):
                        nc.tensor.matmul(pg, lhsT=wg_b[:, cb, :],
                                         rhs=yb[:, cb, scols],
                                         start=(cb == 0), stop=(cb == 1))
                    nc.vector.tensor_scalar_add(g_rep[:, scols], pg, bg_sb)
                # hhT for the j-blocks of this chunk
                for pp in range(DCH // 128):
                    jb = ch * (DCH // 128) + pp
                    ph = prod.tile([128, 512], FP32, tag="prod", name="ph")
                    for cb in range(2):
                        nc.tensor.matmul(ph[:, 0:C],
                                         lhsT=xb[:, cb, ts(jb, 128)],
                                         rhs=wh_b[:, cb, :],
                                         start=(cb == 0), stop=(cb == 1))
                    nc.vector.tensor_copy(hhT[:, jb, 0:C], ph[:, 0:C])

            # ---- main attention loop -----------------------------------
            # Software-pipelined: per group jg of 4 j-blocks, emit logits(jg)
            # and exp(jg), then the UT matmuls of group jg-1 — so the PE works
            # on UT(jg-1) while ACT computes exp(jg).
            NG = NB // 4
            for ic in range(NIC):
                icols = ds(ic * IC, IC)
                uts = [utp.tile([128, C + 1], FP32, tag="ut", name=f"ut{q}")
                       for q in range(NQ)]
                saT = sap.tile([128, 2, IC], BF16, tag="saT")
                prevE = None

                def emit_ut(jg, Es):
                    for q in range(NQ):
                        for b in range(4):
                            j = jg * 4 + b
                            nc.tensor.matmul(
                                uts[q],
                                lhsT=Es[b][:, ds(q * 128, 128)],
                                rhs=hhT[:, j, :],
                                start=(j == 0), stop=(j == NB - 1),
                                skip_group_check=True)

                for jg in range(NG):
                    Es = []
                    for b in range(4):
                        j = jg * 4 + b
                        lt = prod.tile([128, IC], FP32, tag="prod",
                                       name=f"lt{b}")
                        # full-K matmul over the 4 replicated bands => 4*L;
                        # the /4 is folded into the exp's free scale.
                        nc.tensor.matmul(
                            lt,
                            lhsT=g_rep[:, ts(j, 128)],
                            rhs=f_rep[:, icols],
                            start=True, stop=True)
                        E = epool.tile([128, IC], BF16, tag="ex", name=f"ex{b}")
                        nc.scalar.activation(E, lt, AF.Exp, scale=0.25)
                        Es.append(E)
                    if prevE is not None:
                        emit_ut(jg - 1, prevE)
                    prevE = Es
                emit_ut(NG - 1, prevE)
                # tail: normalize, transpose via DMA xbar, residual add, store
                for q in range(NQ):
                    rd = tailp.tile([128, 1], FP32, tag="rd")
                    nc.vector.reciprocal(rd, uts[q][:, C:C + 1])
                    satT = tailp.tile([128, C], BF16, tag="satT")
                    nc.vector.tensor_scalar(satT, uts[q][:, 0:C], rd, gamma_sb,
                                            op0=ALU.mult, op1=ALU.mult)
                    for cb in range(2):
                        nc.sync.dma_start(out=saT[:, cb, ds(q * 128, 128)],
                                          in_=satT[:, ds(cb * 128, 128)],
                                          transpose=True)
                stage = stgp.tile([128, 2, IC], FP32, tag="stage")
                for cb in range(2):
                    nc.vector.tensor_scalar_add(stage[:, cb, :], saT[:, cb, :],
                                                gbh[:, cb, :])
                    nc.vector.tensor_add(stage[:, cb, :], stage[:, cb, :],
                                         x_sb[:, cb, icols])
                    nc.sync.dma_start(out=out_d[cb * 128:(cb + 1) * 128, icols],
                                      in_=stage[:, cb, :])

    nc.compile()
    return nc


_NC_CACHE = {}


def _get_nc(n=4096):
    if n not in _NC_CACHE:
        _NC_CACHE[n] = build_nc(n)
    return _NC_CACHE[n]


def make_in_maps(x, y, Wf, bf, Wg, bg, Wh, bh, gamma):
    x = np.asarray(x, dtype=np.float32)
    y = np.asarray(y, dtype=np.float32)
    B, C_, W_, H_ = x.shape
    n = W_ * H_
    wf_rep = np.ascontiguousarray(np.tile(np.asarray(Wf, np.float32).T, (1, 4)))
    wg_rep = np.ascontiguousarray(np.tile(np.asarray(Wg, np.float32).T, (1, 4)))
    whT = np.ascontiguousarray(np.asarray(Wh, np.float32).T)
    bf_rep = np.ascontiguousarray(np.tile(np.asarray(bf, np.float32), 4).reshape(128, 1))
    bg_rep = np.ascontiguousarray(np.tile(np.asarray(bg, np.float32), 4).reshape(128, 1))
    bh_ = np.asarray(bh, np.float32).reshape(C_, 1)
    gm_ = np.asarray(gamma, np.float32).reshape(1, 1)
    in_maps = []
    for b in range(B):
        in_maps.append({
            "x": np.ascontiguousarray(x[b].reshape(C_, n)),
            "y": np.ascontiguousarray(y[b].reshape(C_, n)),
            "wfT_rep": wf_rep, "wgT_rep": wg_rep, "whT": whT,
            "bf_rep": bf_rep, "bg_rep": bg_rep, "bh": bh_, "gamma": gm_,
        })
    return in_maps, (B, C_, W_, H_)


def run_spmd(inputs: dict, trace: bool = False):
    """Run the SPMD kernel; returns (out [B,C,W,H], BassKernelResults)."""
    in_maps, (B, C_, W_, H_) = make_in_maps(**inputs)
    nc = _get_nc(W_ * H_)
    res = run_bass_kernel_spmd(nc, in_maps, core_ids=list(range(B)), trace=trace)
    out = np.stack([res.results[b]["out"].reshape(C_, W_, H_) for b in range(B)])
    return np.ascontiguousarray(out, dtype=np.float32), res


def kernel(x, y, Wf, bf, Wg, bg, Wh, bh, gamma):
    out, _ = run_spmd(dict(x=x, y=y, Wf=Wf, bf=bf, Wg=Wg, bg=bg,
                           Wh=Wh, bh=bh, gamma=gamma))
    return out
